# revision 44
# baseline (speedup 1.0000x reference)
"""Trainium2 Bass kernel for nn_BasicBlock (binary-activation conv block).

Reference forward (per element):
    act  = sign(x + b0)                      # {-1, 0, +1}
    bw   = scale_c * sign(w),  scale_c = mean|w| over (ci,kh,kw)
    raw  = conv3x3(act, sign(w))             # exact small integers
    y    = (scale*raw - mu) * rsqrt(var + eps) * gamma + beta + x + b1
    out  = prelu(y, alpha) + b2
with BN stats (mu, var) over the FULL batch (sync-BN across cores).

Strategy (8 NeuronCores, batch-sharded 4 imgs/core), SINGLE conv pass:
  - act/weights are {-1,0,+1} -> fp8 matmuls with fp32 PSUM are EXACT.
  - conv: K=128 DoubleRow fp8 matmuls pack FOUR taps per instruction:
    partitions 0-63 carry the act window, partitions 64-127 a row-shifted
    copy (tap +1 row), and the DoubleRow k-subtile pair adds a second
    column/row shift. 9 taps (+3 zero-weight slots) = 3 matmuls per
    row-chunk per image at ~230-350 ns each (measured).
  - DoubleRow psum dst must start at partition 0, so all four images
    accumulate on psum partitions 0-63 (one bank each); img1/img3's raw
    is staged and DMA'd to SBUF partitions 64-127 to realign with x.
  - raw conv output (small integers, |raw| <~ 120 << 256) is stored as
    bf16 EXACTLY; bn stats are computed from it (also exact).
  - sync-BN: tiny AllGather of per-channel (sum, sumsq) + local reduce.
  - epilogue: tmp = A*raw + x (DVE), out = Prelu(tmp + B) in place (ACT),
    DMA out; A = gamma*scale*rsqrt(var+eps), B = beta + b1 - mu*A.

kernel(**inputs) takes FULL inputs, shards, runs SPMD on cores 0-7, gathers.
"""
import numpy as np
from contextlib import ExitStack

import bass_rust
from concourse import bacc, mybir, tile
from concourse.tile_rust import add_dep_helper
from concourse.bass_utils import run_bass_kernel_spmd

# ---------------- problem constants (hardcoded per spec) ----------------
N_CORES = 8
IMGS = 4          # images per core
C = 64            # channels
H = W = 112
HP = WP = 114     # zero-padded act dims
BN_EPS = 1e-5
NG = 32 * H * W   # global BN count per channel

f32 = mybir.dt.float32
bf16 = mybir.dt.bfloat16
fp8 = mybir.dt.float8e4

RPC = 4            # output rows per psum bank
NCHUNK = H // RPC  # 28 row-chunks
EBLK = 4           # chunks per epilogue block (16 rows)

# DR128 conv ops: (wslot, ky, kx, d_s) -> one DoubleRow matmul covering taps
#   (ky,kx)        on partitions 0-63,  subtile 0
#   (ky+1,kx)      on partitions 64-127, subtile 0   (row-shifted act copy)
#   (ky,kx)+d_s    on partitions 0-63,  subtile 1
#   (ky+1,kx)+d_s  on partitions 64-127, subtile 1
# d_s must be EVEN (odd subtile strides fault at runtime).
CONV_OPS = [
    (0, 0, 0, 2),      # taps (0,0) (1,0) (0,2) (1,2)
    (1, 0, 1, 228),    # taps (0,1) (1,1) (2,1) zero
    (2, 2, 0, 2),      # taps (2,0) zero  (2,2) zero
]

# x DMA row blocks (strictly chained per slot, one per band)
XBLK = [(0, 29), (29, 57), (57, 85), (85, 112)]
# sign slices per band: (a0, a1) inclusive padded-act-row ranges
SIGN_SLICES = {
    0: [(1, 29)],
    1: [(28, 57)],
    2: [(56, 85)],
    3: [(84, 112)],
}


def build_program(with_b2: bool):
    nc = bacc.Bacc("TRN2", target_bir_lowering=False, debug=False,
                   num_devices=N_CORES)

    x_d = nc.dram_tensor("x", [IMGS, C, H, W], f32, kind="ExternalInput")
    b0_d = nc.dram_tensor("b0", [1, C, 1, 1], f32, kind="ExternalInput")
    w_d = nc.dram_tensor("w", [C, C, 3, 3], f32, kind="ExternalInput")
    gamma_d = nc.dram_tensor("gamma", [C], f32, kind="ExternalInput")
    beta_d = nc.dram_tensor("beta", [C], f32, kind="ExternalInput")
    b1_d = nc.dram_tensor("b1", [1, C, 1, 1], f32, kind="ExternalInput")
    alpha_d = nc.dram_tensor("alpha", [C], f32, kind="ExternalInput")
    b2_d = nc.dram_tensor("b2", [1, C, 1, 1], f32, kind="ExternalInput")
    ident_d = nc.dram_tensor("ident", [64, 64], f32, kind="ExternalInput")
    out_d = nc.dram_tensor("out", [IMGS, C, H, W], f32, kind="ExternalOutput")

    AF = mybir.ActivationFunctionType
    OP = mybir.AluOpType
    PM = mybir.MatmulPerfMode

    with tile.TileContext(nc) as tc, ExitStack() as ctx:
        pool = ctx.enter_context(tc.tile_pool(name="sbuf", bufs=1))
        dram = ctx.enter_context(tc.tile_pool(name="dram", bufs=1, space="DRAM"))
        conv_ctx = ExitStack()
        actp = conv_ctx.enter_context(tc.tile_pool(name="acts", bufs=2))
        stgp = conv_ctx.enter_context(tc.tile_pool(name="stage", bufs=2))
        rsp = conv_ctx.enter_context(tc.tile_pool(name="rawstg", bufs=4))

        # ---------------- small params (scalar queue) ----------------
        par = pool.tile([64, 6], f32)
        nc.scalar.dma_start(par[:, 0:1], b0_d.ap().rearrange("a c e f -> (a c) (e f)"))
        nc.scalar.dma_start(par[:, 1:2], gamma_d.ap().rearrange("c -> c ()"))
        nc.scalar.dma_start(par[:, 2:3], beta_d.ap().rearrange("c -> c ()"))
        nc.scalar.dma_start(par[:, 3:4], b1_d.ap().rearrange("a c e f -> (a c) (e f)"))
        nc.scalar.dma_start(par[:, 4:5], alpha_d.ap().rearrange("c -> c ()"))
        nc.scalar.dma_start(par[:, 5:6], b2_d.ap().rearrange("a c e f -> (a c) (e f)"))
        rep = pool.tile([128, 6], f32)
        nc.vector.tensor_copy(rep[0:64, :], par[:])
        nc.scalar.dma_start(rep[64:128, :], rep[0:64, :])
        b0_ap = rep[:, 0:1]
        gamma_ap = rep[:, 1:2]
        beta_ap = rep[:, 2:3]
        b1_ap = rep[:, 3:4]
        alpha_ap = rep[:, 4:5]
        b2_ap = rep[:, 5:6]

        # ---------------- all gpsimd memsets BEFORE the x chain occupies
        # the gpsimd engine (its chained x triggers stall the engine) -----
        zt = pool.tile([64, IMGS, WP], fp8)   # zero source for pad DMAs
        nc.gpsimd.memset(zt[:], 0.0)
        eps_t = pool.tile([128, 1], f32)
        nc.gpsimd.memset(eps_t[:], BN_EPS)
        w_dr = pool.tile([128, 3, 2, 64], fp8)
        nc.gpsimd.memset(w_dr[64:128, 1:3, 1, :], 0.0)  # op2 hi-sub, op3 hi-sub
        nc.gpsimd.memset(w_dr[64:128, 2, 0, :], 0.0)    # op3 hi
        # pre-create the two stage buffers and zero their col pads once;
        # sign writes never touch cols 0/113, so the pads persist.
        for _ in range(2):
            stg0 = stgp.tile([128, 30, WP], fp8, tag="stg")
            nc.gpsimd.memset(stg0[:, :, 0:114:113], 0.0)

        # warm the collective path early (async; result unused)
        wg_in = dram.tile([64, 2], f32)
        wg_out = dram.tile([64 * N_CORES, 2], f32)
        nc.sync.dma_start(wg_in[:], par[:, 0:2])
        nc.gpsimd.collective_compute(
            "AllGather", OP.bypass, ins=[wg_in.opt()], outs=[wg_out.opt()],
            replica_groups=[list(range(N_CORES))])

        # ---------------- x load: two parallel per-slot chains ----------------
        # blocks 0-1 are strictly chained so the first bands land early; the
        # tail blocks chain onto block 1 and then fair-share, so the trigger
        # engines are free for other DMAs well before x-in completes.
        x_sb = pool.tile([128, 2, H, W], f32)
        x_v = x_d.ap().rearrange("i c h w -> (i c) h w")
        prev = {0: None, 1: None}
        for bi, (r0, r1) in enumerate(XBLK):
            for s, eng in ((0, nc.sync), (1, nc.gpsimd)):
                ins = eng.dma_start(x_sb[:, s, r0:r1, :],
                                    x_v[128 * s:128 * (s + 1), r0:r1, :])
                if prev[s] is not None:
                    add_dep_helper(ins.ins, prev[s].ins,
                                   reason="serialize x blocks per slot")
                prev[s] = ins

        # ---------------- weight prep (on device) ----------------
        scale128 = pool.tile([128, 1], f32)
        # w_taps9[ci, t, co]: transposed sign weights, taps t = ky*3+kx
        w_taps9 = pool.tile([64, 9, 64], fp8)
        with tc.tile_pool(name="wprep", bufs=1) as wprep, \
                tc.tile_pool(name="wps", bufs=2, space="PSUM") as wps:
            w_sb = wprep.tile([64, 576], f32)
            nc.scalar.dma_start(w_sb[:], w_d.ap().rearrange("o i kh kw -> o (i kh kw)"))
            nc.vector.tensor_reduce(scale128[0:64, :], w_sb[:],
                                    axis=mybir.AxisListType.X,
                                    op=OP.add, apply_absolute_value=True)
            nc.vector.tensor_scalar(scale128[0:64, :], scale128[0:64, :],
                                    1.0 / 576.0, None, op0=OP.mult)
            nc.scalar.dma_start(scale128[64:128, :], scale128[0:64, :])
            # sgn(w) = 2*(w>0)-1 in place (matches reference incl. w==0 -> -1)
            nc.vector.tensor_scalar(w_sb[:], w_sb[:], 0.0, None, op0=OP.is_gt)
            nc.vector.tensor_scalar(w_sb[:], w_sb[:], 2.0, -1.0,
                                    op0=OP.mult, op1=OP.add)
            sgnw_bf = wprep.tile([64, 576], bf16)
            nc.vector.tensor_copy(sgnw_bf[:], w_sb[:])
            ident_f = wprep.tile([64, 64], f32)
            nc.scalar.dma_start(ident_f[:], ident_d[:])
            ident_bf = wprep.tile([64, 64], bf16)
            nc.vector.tensor_copy(ident_bf[:], ident_f[:])
            sgn_view = sgnw_bf[:].rearrange("o (i t) -> o t i", t=9)
            for t in range(9):
                wtp = wps.tile([64, 64], bf16, tag="wt")
                nc.tensor.transpose(wtp[:], sgn_view[:, t, :], ident_bf[:])
                nc.vector.tensor_copy(w_taps9[:, t, :], wtp[:])

        # w_dr[128, op, subtile, co]: DoubleRow weights per CONV_OPS entry.
        T = lambda ky, kx: ky * 3 + kx
        # lower-partition copies (DVE, same partitions)
        nc.vector.tensor_copy(w_dr[0:64, 0, 0, :], w_taps9[:, T(0, 0), :])
        nc.vector.tensor_copy(w_dr[0:64, 0, 1, :], w_taps9[:, T(0, 2), :])
        nc.vector.tensor_copy(w_dr[0:64, 1, 0, :], w_taps9[:, T(0, 1), :])
        nc.vector.tensor_copy(w_dr[0:64, 1, 1, :], w_taps9[:, T(2, 1), :])
        nc.vector.tensor_copy(w_dr[0:64, 2, 0, :], w_taps9[:, T(2, 0), :])
        nc.vector.tensor_copy(w_dr[0:64, 2, 1, :], w_taps9[:, T(2, 2), :])
        # upper-partition copies (DMA, crossing partitions)
        nc.scalar.dma_start(w_dr[64:128, 0, 0, :], w_taps9[:, T(1, 0), :])
        nc.scalar.dma_start(w_dr[64:128, 0, 1, :], w_taps9[:, T(1, 2), :])
        nc.scalar.dma_start(w_dr[64:128, 1, 0, :], w_taps9[:, T(1, 1), :])

        # precomputed epilogue params
        gs = pool.tile([128, 1], f32)      # gamma * scale
        nc.vector.tensor_tensor(gs[:], gamma_ap, scale128[:], op=OP.mult)
        bb1 = pool.tile([128, 1], f32)     # beta + b1
        nc.vector.tensor_tensor(bb1[:], beta_ap, b1_ap, op=OP.add)
        sc2 = pool.tile([128, 1], f32)     # scale^2
        nc.vector.tensor_tensor(sc2[:], scale128[:], scale128[:], op=OP.mult)

        # ---------------- storage for raw conv output & stats ----------------
        raw = pool.tile([128, 2, H, W], bf16)
        bnst = pool.tile([128, 2 * NCHUNK, 6], f32)

        psum = ctx.enter_context(tc.tile_pool(name="psum", bufs=2, space="PSUM"))

        def make_band(b):
            """Act band b: [128, img, 30, 114] fp8; lower partitions hold
            padded act rows [28b, 28b+29], upper partitions the same data
            shifted UP one row (upper row r = act row 28b+r+1). Pads come
            in through the stage's pre-zeroed cols; row pads are DMA'd
            from the zero tile on the scalar queue."""
            ab_t = actp.tile([128, IMGS, 30, WP], fp8, tag="act",
                             name=f"act{b}")
            if b <= 1:
                # upper row 29 is never written by distribution DMAs
                nc.scalar.dma_start(ab_t[64:128, :, 29:30, :], zt[:])
            if b == 0:
                nc.scalar.dma_start(ab_t[0:64, :, 0:1, :], zt[:])
            if b == 3:
                nc.scalar.dma_start(ab_t[0:64, :, 29:30, :], zt[:])
            last_sign = None
            # everything act-related flows on the scalar queue: the sync and
            # gpsimd engines are stalled walking their chained x blocks
            deng = nc.scalar
            for (a0, a1) in SIGN_SLICES[b]:
                nr = a1 - a0 + 1
                l0 = a0 - 28 * b
                xr = a0 - 1
                # upper-half rows [u0, l0+nr-2] <- stage rows [sk, ...]
                u0 = max(l0 - 1, 0)
                sk = u0 - (l0 - 1)
                nrc = l0 + nr - 1 - u0
                for s in range(2):
                    iA, iB = 2 * s, 2 * s + 1
                    stg = stgp.tile([128, 30, WP], fp8, tag="stg")
                    last_sign = nc.scalar.activation(
                        stg[:, 0:nr, 1:113], x_sb[:, s, xr:xr + nr, :],
                        AF.Sign, bias=b0_ap)
                    deng.dma_start(ab_t[0:64, iA, l0:l0 + nr, :],
                                   stg[0:64, 0:nr, :])
                    deng.dma_start(ab_t[0:64, iB, l0:l0 + nr, :],
                                   stg[64:128, 0:nr, :])
                    deng.dma_start(ab_t[64:128, iA, u0:u0 + nrc, :],
                                   stg[0:64, sk:sk + nrc, :])
                    deng.dma_start(ab_t[64:128, iB, u0:u0 + nrc, :],
                                   stg[64:128, sk:sk + nrc, :])
            return ab_t, last_sign

        def dr_view(band, img, lrow, ky, kx, d_s):
            """[128, 2, 4, 112] moving AP: subtile i reads the act window
            shifted by i*d_s elements (both partition halves)."""
            base = band[:, img, lrow + ky:lrow + ky + 4, kx:kx + 112]
            v = base.copy()
            l = base.ap.to_list()
            v.ap = bass_rust.VecI64Pair([l[0], [d_s, 2], l[1], l[2]])
            return v

        def conv_chunk(pt, band, r):
            """9-tap binary conv for row-chunk r of all 4 images: 3 DoubleRow
            fp8 matmuls per image into its own psum bank (partitions 0-63)."""
            lrow = (r - 7 * (r // 7)) * RPC
            for o, (ws, ky, kx, d_s) in enumerate(CONV_OPS):
                for img in range(IMGS):
                    rhs = dr_view(band, img, lrow, ky, kx, d_s)
                    dst = pt[0:64, img, 0:448].rearrange(
                        "p (r c) -> p r c", r=4)
                    nc.tensor.matmul(
                        dst, w_dr[:, ws, :, :], rhs,
                        start=(o == 0), stop=(o == 2),
                        perf_mode=PM.DoubleRow, tile_position=(0, 0))

        # ---------------- conv + stats (single pass) ----------------
        pend_stats = []

        def emit_stats(r0_):
            for rr in (r0_, r0_ + 1):
                rws = slice(rr * RPC, (rr + 1) * RPC)
                for s in range(2):
                    nc.vector.bn_stats(
                        bnst[:, 2 * rr + s, :],
                        raw[:, s, rws, :].rearrange("p r c -> p (r c)"))

        last_sign_ins = None
        rst = None
        for b in range(4):
            band, last_sign_ins = make_band(b)
            for r in range(7 * b, 7 * b + 7):
                pt = psum.tile([64, IMGS, 512], f32, tag="cv")
                conv_chunk(pt, band, r)
                rows = slice(r * RPC, (r + 1) * RPC)
                # img0/img2 -> raw lower partitions directly (alternate the
                # copy between ACT and DVE to balance engine load)
                lo_src = pt[0:64, 0:3:2, 0:448].rearrange(
                    "p s (r c) -> p s r c", r=4)
                if r % 2 == 0:
                    nc.scalar.activation(raw[0:64, :, rows, :], lo_src, AF.Copy)
                else:
                    nc.vector.tensor_copy(raw[0:64, :, rows, :], lo_src)
                # img1/img3 -> bf16 staging (DVE); DMA'd to raw upper
                # partitions once per chunk-pair
                if r % 2 == 0:
                    rst = rsp.tile([64, 2, 2 * RPC, W], bf16, tag="rst")
                g = (r % 2) * RPC
                nc.vector.tensor_copy(
                    rst[:, :, g:g + RPC, :],
                    pt[0:64, 1:4:2, 0:448].rearrange("p s (r c) -> p s r c", r=4))
                if r % 2 == 1:
                    nc.scalar.dma_start(
                        raw[64:128, :, (r - 1) * RPC:(r + 1) * RPC, :], rst[:])
                    # full-width bn stats need the upper-partition DMA; emit
                    # them 3 pairs late so the DVE FIFO never blocks on it
                    pend_stats.append(r - 1)
                    if len(pend_stats) > 3:
                        emit_stats(pend_stats.pop(0))

        for p_ in pend_stats:
            emit_stats(p_)

        # conv-phase pools are done: release their SBUF for the epilogue
        conv_ctx.close()

        # warm ACT tables for Sqrt/Prelu so the post-collective chain and
        # epilogue don't pay table loads on the critical path
        scrap = pool.tile([128, 2], f32)
        wi = nc.scalar.activation(scrap[:, 0:1], scale128[:], AF.Sqrt)
        if last_sign_ins is not None:
            add_dep_helper(wi.ins, last_sign_ins.ins, reason="warm after signs")
        nc.scalar.activation(scrap[:, 1:2], scale128[:], AF.Prelu,
                             bias=0.0, scale=1.0, alpha=0.25)

        # ---------------- sync-BN: AllGather (sum, sumsq), local reduce -----
        mv = pool.tile([128, 2], f32)
        nc.vector.bn_aggr(mv[:], bnst[:])
        NL = float(IMGS // 2 * H * W)  # elements per partition (2 imgs)
        ssq = pool.tile([128, 2], f32)
        nc.vector.tensor_scalar(ssq[:, 0:1], mv[:, 0:1], NL, None, op0=OP.mult)
        nc.vector.scalar_tensor_tensor(ssq[:, 1:2], mv[:, 0:1], mv[:, 0:1],
                                       mv[:, 1:2], op0=OP.mult, op1=OP.add)
        nc.vector.tensor_scalar(ssq[:, 1:2], ssq[:, 1:2], NL, None, op0=OP.mult)

        ag_in = dram.tile([128, 2], f32)
        ag_out = dram.tile([128 * N_CORES, 2], f32)
        nc.sync.dma_start(ag_in[:], ssq[:])
        nc.gpsimd.collective_compute(
            "AllGather", OP.bypass, ins=[ag_in.opt()], outs=[ag_out.opt()],
            replica_groups=[list(range(N_CORES))])
        # gather [8*128, 2] -> sbuf [128, 2, 8] and reduce over cores
        gath = pool.tile([128, 2, N_CORES], f32)
        nc.sync.dma_start(gath[:],
                          ag_out[:].rearrange("(k p) s -> p s k", k=N_CORES))
        g_sb = pool.tile([128, 2], f32)
        nc.vector.tensor_reduce(g_sb[:], gath[:], axis=mybir.AxisListType.X,
                                op=OP.add)
        # combine partition halves on BOTH halves (no post-math broadcast)
        swap = pool.tile([128, 2], f32)
        nc.sync.dma_start(swap[0:64, :], g_sb[64:128, :])
        nc.sync.dma_start(swap[64:128, :], g_sb[0:64, :])
        tot = pool.tile([128, 2], f32)
        nc.vector.tensor_tensor(tot[:], g_sb[:], swap[:], op=OP.add)

        # ---------------- A, B computation (128-wide) ----------------
        mean_g = pool.tile([128, 1], f32)
        nc.vector.tensor_scalar(mean_g[:], tot[:, 0:1], 1.0 / NG, None, op0=OP.mult)
        ex2 = pool.tile([128, 1], f32)
        nc.vector.tensor_scalar(ex2[:], tot[:, 1:2], 1.0 / NG, None, op0=OP.mult)
        m2 = pool.tile([128, 1], f32)
        nc.vector.tensor_tensor(m2[:], mean_g[:], mean_g[:], op=OP.mult)
        var_r = pool.tile([128, 1], f32)
        nc.vector.tensor_tensor(var_r[:], ex2[:], m2[:], op=OP.subtract)
        vpe = pool.tile([128, 1], f32)
        nc.vector.scalar_tensor_tensor(vpe[:], var_r[:], sc2[:], eps_t[:],
                                       op0=OP.mult, op1=OP.add)
        sq = pool.tile([128, 1], f32)
        nc.scalar.activation(sq[:], vpe[:], AF.Sqrt)
        r0_t = pool.tile([128, 1], f32)
        nc.vector.reciprocal(r0_t[:], sq[:])
        rr = pool.tile([128, 1], f32)
        nc.vector.tensor_tensor(rr[:], r0_t[:], r0_t[:], op=OP.mult)
        nc.vector.tensor_tensor(rr[:], rr[:], vpe[:], op=OP.mult)
        nc.vector.tensor_scalar(rr[:], rr[:], -0.5, 1.5, op0=OP.mult, op1=OP.add)
        rsq = pool.tile([128, 1], f32)
        nc.vector.tensor_tensor(rsq[:], r0_t[:], rr[:], op=OP.mult)
        ab_p = pool.tile([128, 2], f32)
        nc.vector.tensor_tensor(ab_p[:, 0:1], rsq[:], gs[:], op=OP.mult)
        mA = pool.tile([128, 1], f32)
        nc.vector.tensor_tensor(mA[:], mean_g[:], ab_p[:, 0:1], op=OP.mult)
        nc.vector.tensor_tensor(ab_p[:, 1:2], bb1[:], mA[:], op=OP.subtract)
        A_ap = ab_p[:, 0:1]
        B_ap = ab_p[:, 1:2]

        # ---------------- epilogue (per-slot pipeline) ----------------
        tmpp = ctx.enter_context(tc.tile_pool(name="tmp", bufs=5))
        out_v = out_d.ap().rearrange("i c h w -> (i c) h w")
        dma_engines = [nc.sync, nc.gpsimd, nc.scalar]
        NR = EBLK * RPC  # rows per epilogue block
        for blk in range(NCHUNK // EBLK):
            r0 = blk * NR
            for s in range(2):
                tmp = tmpp.tile([128, NR, W], f32, tag="tmp")
                # tmp = A*raw + x
                nc.vector.scalar_tensor_tensor(
                    tmp[:], raw[:, s, r0:r0 + NR, :], A_ap,
                    x_sb[:, s, r0:r0 + NR, :], op0=OP.mult, op1=OP.add)
                # prelu in place
                nc.scalar.activation(tmp[:], tmp[:], AF.Prelu, bias=B_ap,
                                     scale=1.0, alpha=alpha_ap)
                if with_b2:
                    nc.vector.tensor_scalar(tmp[:], tmp[:], b2_ap, None,
                                            op0=OP.add)
                dst = out_v[128 * s:128 * (s + 1), r0:r0 + NR, :]
                eng = dma_engines[(2 * blk + s) % len(dma_engines)]
                eng.dma_start(dst, tmp[:])

    nc.compile()
    return nc


_CACHE = {}


def _get_program(with_b2: bool):
    if with_b2 not in _CACHE:
        _CACHE[with_b2] = build_program(with_b2)
    return _CACHE[with_b2]


def run_sharded(inputs: dict, trace: bool = False, tmpdir=None):
    """Shard, run on 8 cores, gather. Returns (out, BassKernelResults)."""
    x = np.ascontiguousarray(np.asarray(inputs["x"], dtype=np.float32))
    w = np.ascontiguousarray(np.asarray(inputs["w"], dtype=np.float32))
    b0 = np.ascontiguousarray(np.asarray(inputs["b0"], dtype=np.float32))
    gamma = np.ascontiguousarray(np.asarray(inputs["gamma"], dtype=np.float32))
    beta = np.ascontiguousarray(np.asarray(inputs["beta"], dtype=np.float32))
    b1 = np.ascontiguousarray(np.asarray(inputs["b1"], dtype=np.float32))
    alpha = np.ascontiguousarray(np.asarray(inputs["alpha"], dtype=np.float32))
    b2 = np.ascontiguousarray(np.asarray(inputs["b2"], dtype=np.float32))
    with_b2 = bool(np.any(b2 != 0.0))
    nc = _get_program(with_b2)

    ident = np.eye(64, dtype=np.float32)
    in_maps = []
    for k in range(N_CORES):
        in_maps.append({
            "x": np.ascontiguousarray(x[IMGS * k:IMGS * (k + 1)]),
            "w": w, "b0": b0, "gamma": gamma, "beta": beta, "b1": b1,
            "alpha": alpha, "b2": b2, "ident": ident,
        })
    res = run_bass_kernel_spmd(nc, in_maps, list(range(N_CORES)),
                               trace=trace, tmpdir=tmpdir)
    out = np.concatenate([res.results[k]["out"] for k in range(N_CORES)], axis=0)
    return out, res


def kernel(**inputs) -> np.ndarray:
    out, _ = run_sharded(inputs, trace=False)
    return out
